# revision 33
# baseline (speedup 1.0000x reference)
"""Single-head attention (no 1/sqrt(d) scaling) for Trainium2, 8 NeuronCores.

Problem: x [8, 2048, 768], W [2304, 768], b [2304]
    qkv = x @ W.T + b ; q,k,v = split(qkv)
    out = softmax(q @ k.T) @ v            -> [8, 2048, 768] fp32

Sharding: data-parallel over batch, one batch element per core. Inputs are
host-transposed (xT [768,2048], wT [768,2304]); the kernel emits out^T
[768, 2048] and the host transposes back during the gather.

Best configuration (BEST_KW): every matmul operand in bf16 (x and W ship as
bf16 from the host; k/q/v/P evictions round to bf16; PSUM accumulation stays
fp32). Measured end-to-end rel err 1.44e-2 vs the 2e-2 gate — matches an
f64 rounding study exactly, so the error is deterministic bf16 quantization,
not HW noise. bf16 beats fp32r because fp32r matmuls self-load their
stationary operand at ~2x cost and stream no faster (measured 553us fp32r vs
454us bf16 for the same structure). 512-wide matmuls are issued as 2x256
halves (split=2): HW-measured per-column cost is 0.70ns at 256-wide vs
0.83ns at 512-wide (LDWEIGHTS serializes per matmul; walrus --enable-ldw-opt
rejects our LDWs because they carry semaphore waits, so it cannot be hidden).
The kernel runs at the hardware's practical bf16 matmul-stream rate: 648k
streamed columns x 0.70ns = ~454us/core, vs a 270us ideal-PE floor.

Phase A (k/v projection), looped over 512-wide n-slices of x streaming
through double-buffered SBUF slots, W resident:
    kT = (x @ Wk.T + bk).T  transposed layout [h, n] (lhsT = W block, rhs =
         xT slice; per-partition bias via the ACT eviction). Resident.
    v  = x @ Wv.T + bv      natural layout (lhsT = xT block, rhs = W slice;
         bias via a partition-broadcast DVE add at eviction). Resident.
Phase B (attention) per 512-wide n-slice; no max subtraction (|logits| <~60
<< 88 so exp stays within fp32 range; denominators handled unnormalized):
    qT strip = (x @ Wq.T + bq).T  projected on the fly (no spill round-trip)
    S^T[m,n] = k qT       (PSUM, 6 accumulating matmuls per m-chunk)
    P = exp(S^T)          (ACT, rounds to fp32r)
    U^T += v_m^T @ P      (6 matmuls, accumulated over 16 m-chunks in 6 banks)
    r   += ones128 @ P    (replicated denominator, DVE-accumulated in SBUF)
    out^T slice = U^T * (1/r)  (DVE scale at eviction, DMA straight to DRAM)
The m-loop is software-pipelined (S/exp for chunk i issued ahead of r/U for
chunk i-1) so the PE never waits on the exp; S tiles double-buffer through
2 PSUM banks, U holds 6 banks.
"""

import contextlib

import numpy as np

import concourse.bacc as bacc
import concourse.mybir as mybir
import concourse.tile as tile
from concourse.bass_utils import run_bass_kernel_spmd

F32 = mybir.dt.float32
F32R = mybir.dt.float32r
AF = mybir.ActivationFunctionType
ALU = mybir.AluOpType

B, N, H = 8, 2048, 768
H3 = 3 * H
P = 128
ND = H // P      # 6 d-chunks
NM = N // P      # 16 m-chunks
SL = 512         # n-slice width (fp32 moving-operand max / one PSUM bank)
NSL = N // SL    # 4 n-slices


def build_nc(loop_iters=None, split=1, nm_eff=NM, nsl_eff=NSL, b_off=False, no_r=False, copy_exp=False, pv_bf16=False, all_bf16=False, STORE_GP=True, SPILL_GP=False, vps_bufs=2, p_bufs=4, pv_split=None):
    """Build the attention kernel. loop_iters wraps the whole body in an
    on-device For_i loop (benchmarking only — amortizes dispatch overhead).
    split=2 issues every N=512 matmul as two N=256 halves (same PSUM bank,
    one accumulation group) — empirically faster moving-operand streaming.
    all_bf16: every matmul operand in bf16 (x, W shipped bf16; k/q/v/P
    evictions bf16). Stationary loads become separate LDWEIGHTS that the PE
    reorder window hides. Measured end-to-end rel err ~1e-2 (gate 2e-2)."""
    HS = SL // split  # matmul moving width
    nc = bacc.Bacc("TRN2", target_bir_lowering=False, debug=False)

    BF16 = mybir.dt.bfloat16
    mdt = BF16 if all_bf16 else F32R  # matmul operand dtype (x/W/q/k)
    if all_bf16:
        pv_bf16 = True

    xT = nc.dram_tensor("xT", [H, N], mdt, kind="ExternalInput")
    wT = nc.dram_tensor("wT", [H, H3], mdt, kind="ExternalInput")
    bcol = nc.dram_tensor("bcol", [P, 2 * ND], F32, kind="ExternalInput")
    bvrow = nc.dram_tensor("bvrow", [1, H], F32, kind="ExternalInput")
    out = nc.dram_tensor("out", [H, N], F32, kind="ExternalOutput")  # transposed; host fixes layout


    def mm_group(psum, lhs_list, rhs_slicer, extra=None, split=1):
        """Accumulating matmul group into `psum` [P, SL-or-less wide].

        lhs_list: per-c stationary APs; rhs_slicer(c, lo, w): moving AP slice.
        extra: optional (lhsT, rhs_slicer) K=1 bias pair appended to the group.
        """
        width = psum.shape[-1]
        hw = width // split
        n = len(lhs_list)
        first, last = True, None
        steps = []
        for c in range(n):
            for h in range(split):
                steps.append(("mm", c, h))
        if extra is not None:
            for h in range(split):
                steps.append(("extra", 0, h))
        for idx, (kind, c, h) in enumerate(steps):
            stop = idx == len(steps) - 1
            lo = h * hw
            if kind == "mm":
                nc.tensor.matmul(
                    psum[:, lo : lo + hw], lhs_list[c], rhs_slicer(c, lo, hw),
                    start=(idx == 0), stop=stop,
                )
            else:
                elh, ers = extra
                nc.tensor.matmul(
                    psum[:, lo : lo + hw], elh, ers(0, lo, hw),
                    start=False, stop=stop,
                )

    with tile.TileContext(nc) as tc:
        with (
            tc.tile_pool(name="dram", bufs=1, space="DRAM") as dram,
            tc.tile_pool(name="const", bufs=1) as const,
            tc.tile_pool(name="keep", bufs=1) as keep,
            tc.For_i(0, loop_iters, 1) if loop_iters else contextlib.nullcontext(),
        ):
            bcol_sb = const.tile([P, 2 * ND], F32)
            nc.sync.dma_start(bcol_sb[:], bcol.ap())

            pdt = BF16 if pv_bf16 else F32R
            ones128 = const.tile([P, P], pdt)  # lhsT for the replicated-r matmul
            ones_f32, ones_free = tc.tile([P, P], F32, name="ones_f32")
            nc.gpsimd.memset(ones_f32[:], 1.0)
            nc.scalar.copy(ones128[:], ones_f32[:])
            ones_free()

            # resident across phases
            ktsb = [keep.tile([P, N], mdt, name=f"kT{c}") for c in range(ND)]
            vsb = [keep.tile([P, H], pdt, name=f"v{ni}") for ni in range(NM)]

            with tc.tile_pool(name="xw_pool", bufs=1) as xw:
                # W resident. q/k sections as [128,128] h-slices so compute
                # unlocks at DMA-stream granularity; v as [128, 768].
                HH = H // 2
                wq = [
                    [xw.tile([P, HH], mdt, name=f"wq{c}_{h}") for h in range(2)]
                    for c in range(ND)
                ]

                def wslice(blks, c, hc):
                    half, col = divmod(hc * P, HH)
                    return blks[c][half][:, col : col + P]
                xwa = tc.alloc_tile_pool(name="xwa_pool", bufs=1)
                wk = [
                    [xwa.tile([P, HH], mdt, name=f"wk{c}_{h}") for h in range(2)]
                    for c in range(ND)
                ]
                wv = [xwa.tile([P, H], mdt, name=f"wv{c}") for c in range(ND)]
                # x slices stream through 2 slots per d-chunk; every load
                # allocates fresh tiles so the tag rotation stays consistent
                xts = {}

                def fresh_xt(s, phase):
                    tiles = [
                        xw.tile([P, SL], mdt, name=f"xt{phase}{c}_{s}",
                                tag=f"xt{c}", bufs=2)
                        for c in range(ND)
                    ]
                    for c in range(ND):
                        nc.sync.dma_start(
                            tiles[c][:],
                            xT.ap()[c * P : (c + 1) * P, s * SL : (s + 1) * SL],
                        )
                    xts[s] = tiles
                    return tiles

                def load_w_half(blks, lo, h):
                    for c in range(ND):
                        nc.sync.dma_start(
                            blks[c][h][:],
                            wT.ap()[c * P : (c + 1) * P, lo + h * HH : lo + (h + 1) * HH],
                        )

                bvb = xwa.tile([P, H], F32, name="bvb")
                nc.sync.dma_start(bvb[:1, :], bvrow.ap())
                nc.gpsimd.partition_broadcast(bvb[:], bvb[:1, :])

                # DMA order = compute-unlock order: phase A starts with the
                # k projection, so k weights + x slice 0 first; wq (only
                # needed in phase B) last.
                load_w_half(wk, H, 0)
                fresh_xt(0, "a")
                load_w_half(wk, H, 1)
                for c in range(ND):
                    nc.sync.dma_start(
                        wv[c][:], wT.ap()[c * P : (c + 1) * P, 2 * H : 3 * H]
                    )
                fresh_xt(1, "a")
                load_w_half(wq, 0, 0)
                load_w_half(wq, 0, 1)

                with (
                    tc.tile_pool(name="qkps", bufs=3, space="PSUM") as qkps,
                    tc.tile_pool(name="vps", bufs=vps_bufs, space="PSUM") as vps,
                ):
                    for ns in range(NSL):
                        ssl = slice(ns * SL, (ns + 1) * SL)
                        if ns >= 1 and ns + 1 < NSL:
                            fresh_xt(ns + 1, "a")

                        # --- k projection for this slice (resident) ---
                        for hc in range(ND):
                            ps = qkps.tile([P, SL], F32, name="qkpsum", tag="qk")
                            mm_group(
                                ps, [wslice(wk, c, hc) for c in range(ND)],
                                lambda c, lo, w, _ns=ns: xts[_ns][c][:, lo : lo + w],
                                split=split,
                            )
                            nc.scalar.activation(
                                ktsb[hc][:, ssl], ps[:], AF.Identity,
                                bias=bcol_sb[:, ND + hc : ND + hc + 1],
                            )

                        # --- v projection for the 4 n-chunks of this slice ---
                        for ni in range(4 * ns, 4 * ns + 4):
                            lsl = slice((ni % NSL) * P, (ni % NSL) * P + P)
                            pa = vps.tile([P, SL], F32, name="pa", tag="pa")
                            pb = vps.tile([P, H - SL], F32, name="pb", tag="pb")
                            mm_group(
                                pa, [xts[ns][c][:, lsl] for c in range(ND)],
                                lambda c, lo, w: wv[c][:, lo : lo + w],
                                split=split,
                            )
                            mm_group(
                                pb, [xts[ns][c][:, lsl] for c in range(ND)],
                                lambda c, lo, w: wv[c][:, SL + lo : SL + lo + w],
                            )
                            nc.vector.tensor_tensor(
                                vsb[ni][:, 0:SL], pa[:], bvb[:, 0:SL], op=ALU.add
                            )
                            nc.vector.tensor_tensor(
                                vsb[ni][:, SL:H], pb[:], bvb[:, SL:H], op=ALU.add
                            )

                for s in range(min(2, nsl_eff)):
                    fresh_xt(s, "b")
                xwa.release()

                if b_off:
                    for c in range(ND):
                        nc.sync.dma_start(
                            out.ap()[c * P : (c + 1) * P, :], ktsb[c][:]
                        )
                # ---- Phase B: attention (software-pipelined m-loop) ----
                with (
                    contextlib.nullcontext() if b_off else contextlib.nullcontext(),
                    tc.tile_pool(name="qsb_pool", bufs=2) as qsb_pool,
                    tc.tile_pool(name="p_pool", bufs=p_bufs) as p_pool,
                    tc.tile_pool(name="u_ps", bufs=1, space="PSUM") as u_ps,
                    tc.tile_pool(name="sps", bufs=2, space="PSUM") as sps,
                    tc.tile_pool(name="usb_pool", bufs=1) as usb_pool,
                    tc.tile_pool(name="misc", bufs=1) as misc,
                ):
                    for ns in range(0 if b_off else nsl_eff):
                        if ns + 2 < nsl_eff:
                            fresh_xt(ns + 2, "b")
                        # project this slice's q strip (transposed layout)
                        qsbuf = []
                        for hc in range(ND):
                            ps = sps.tile([P, SL], F32, name="s_ps", tag="s")
                            mm_group(
                                ps, [wslice(wq, c, hc) for c in range(ND)],
                                lambda c, lo, w, _ns=ns: xts[_ns][c][:, lo : lo + w],
                                split=split,
                            )
                            qc = qsb_pool.tile([P, SL], mdt, name=f"qsb{hc}", tag=f"qsb{hc}")
                            nc.scalar.activation(
                                qc[:], ps[:], AF.Identity, bias=bcol_sb[:, hc : hc + 1]
                            )
                            qsbuf.append(qc)
                        us = [
                            u_ps.tile([P, SL], F32, name=f"u{c}", tag=f"u{c}")
                            for c in range(ND)
                        ]
                        r_sb = misc.tile([P, SL], F32, name="r_sb", tag="r_sb")

                        p_sbs = [None] * NM
                        for mi in range(nm_eff + 1):
                            if mi < nm_eff:
                                msl = slice(mi * P, (mi + 1) * P)
                                s_ps = sps.tile([P, SL], F32, name="s_ps", tag="s")
                                mm_group(
                                    s_ps, [ktsb[c][:, msl] for c in range(ND)],
                                    lambda c, lo, w: qsbuf[c][:, lo : lo + w],
                                    split=split,
                                )
                                p_sb = p_pool.tile([P, SL], pdt, name="p_sb", tag="p")
                                nc.scalar.activation(
                                    p_sb[:], s_ps[:], AF.Copy if copy_exp else AF.Exp
                                )
                                p_sbs[mi] = p_sb
                            if mi >= 1:
                                j = mi - 1
                                pj = p_sbs[j]
                                if not no_r:
                                    r_ps = sps.tile([P, SL], F32, name="r_ps", tag="s")
                                    mm_group(
                                        r_ps, [ones128[:]],
                                        lambda c, lo, w: pj[:, lo : lo + w],
                                        split=split,
                                    )
                                    if j == 0:
                                        nc.vector.tensor_copy(r_sb[:], r_ps[:])
                                    else:
                                        nc.vector.tensor_tensor(
                                            r_sb[:], r_ps[:], r_sb[:], op=ALU.add
                                        )
                                pvs = pv_split if pv_split is not None else split
                                hw2 = SL // pvs
                                for c in range(ND):
                                    for h in range(pvs):
                                        lo = h * hw2
                                        nc.tensor.matmul(
                                            us[c][:, lo : lo + hw2],
                                            vsb[j][:, c * P : (c + 1) * P],
                                            pj[:, lo : lo + hw2],
                                            start=(j == 0 and h == 0),
                                            stop=(j == nm_eff - 1 and h == pvs - 1),
                                        )
                                p_sbs[j] = None

                        rinv = misc.tile([P, SL], F32, name="rinv", tag="rinv")
                        if no_r:
                            nc.vector.tensor_copy(rinv[:], r_sb[:])
                        else:
                            nc.vector.reciprocal(rinv[:], r_sb[:])

                        for c in range(ND):
                            u_sb = usb_pool.tile([P, SL], F32, name=f"usb{c}", tag=f"usb{c}")
                            nc.vector.tensor_tensor(u_sb[:], us[c][:], rinv[:], op=ALU.mult)
                            store_eng = nc.gpsimd if STORE_GP else nc.sync
                            store_eng.dma_start(
                                out.ap()[c * P : (c + 1) * P, ns * SL : (ns + 1) * SL],
                                u_sb[:],
                            )

    nc.compile()
    return nc


def build_nc_v2(loop_iters=None, split=2, nm_eff=NM, nsl_eff=NSL, body_reps=1,
                sorder="c", proj_split=None, dve_evict=False, q_in_a=False):
    """v2: all-bf16 matmul operands, batched DMA, x/W resident in SBUF.

    vs build_nc: x and W ship as bf16 and load with 24 large need-ordered
    DMAs (wk sections -> x slice 0 -> x rest -> wv -> wq) instead of ~60
    small ones; x stays resident so phase B does no reloads; ones/bias
    tiles come from the host instead of a gpsimd memset/broadcast chain.
    """
    nc = bacc.Bacc("TRN2", target_bir_lowering=False, debug=False)
    BF16 = mybir.dt.bfloat16

    xT = nc.dram_tensor("xT", [H, N], BF16, kind="ExternalInput")
    wT = nc.dram_tensor("wT", [H, H3], BF16, kind="ExternalInput")
    bcol = nc.dram_tensor("bcol", [P, 2 * ND], F32, kind="ExternalInput")
    bvb_d = nc.dram_tensor("bvb", [P, H], F32, kind="ExternalInput")
    ones_d = nc.dram_tensor("ones", [P, P], BF16, kind="ExternalInput")
    out = nc.dram_tensor("out", [H, N], F32, kind="ExternalOutput")

    if proj_split is None:
        proj_split = split

    def mm_group(psum, lhs_list, rhs_slicer, split=1):
        width = psum.shape[-1]
        hw = width // split
        if sorder == "h":
            steps = [(c, h) for h in range(split) for c in range(len(lhs_list))]
        else:
            steps = [(c, h) for c in range(len(lhs_list)) for h in range(split)]
        for idx, (c, h) in enumerate(steps):
            lo = h * hw
            nc.tensor.matmul(
                psum[:, lo : lo + hw], lhs_list[c], rhs_slicer(c, lo, hw),
                start=(idx == 0), stop=(idx == len(steps) - 1),
            )

    with tile.TileContext(nc) as tc:
        with (
            tc.tile_pool(name="const", bufs=1) as const,
            tc.tile_pool(name="keep", bufs=1) as keep,
            tc.For_i(0, loop_iters, 1) if loop_iters else contextlib.nullcontext(),
        ):
            bcol_sb = const.tile([P, 2 * ND], F32)
            ones128 = const.tile([P, P], BF16, name="ones128")
            bvb = const.tile([P, H], F32, name="bvb")
            nc.sync.dma_start(bcol_sb[:], bcol.ap())
            nc.sync.dma_start(ones128[:], ones_d.ap())
            nc.sync.dma_start(bvb[:], bvb_d.ap())

            # resident inputs + intermediates
            xts = [keep.tile([P, N], BF16, name=f"x{c}") for c in range(ND)]
            wa = [keep.tile([P, H3], BF16, name=f"w{c}") for c in range(ND)]
            ktsb = [keep.tile([P, N], BF16, name=f"kT{c}") for c in range(ND)]
            vsb = [keep.tile([P, H], BF16, name=f"v{ni}") for ni in range(NM)]
            qres = (
                [keep.tile([P, N], BF16, name=f"qT{c}") for c in range(ND)]
                if q_in_a else None
            )

            def evict_bias(dst, ps, bias_col):
                if dve_evict:
                    nc.vector.tensor_scalar_add(dst, ps, bias_col)
                else:
                    nc.scalar.activation(dst, ps, AF.Identity, bias=bias_col)

            rows = lambda c: slice(c * P, (c + 1) * P)
            wslice = lambda c, base, hc: wa[c][:, base + hc * P : base + (hc + 1) * P]

            for rep in range(body_reps):
              # need-ordered loads: wk, x slice 0, x rest, wv, wq
              for c in range(ND):
                nc.sync.dma_start(wa[c][:, H : 2 * H], wT.ap()[rows(c), H : 2 * H])
              for c in range(ND):
                nc.sync.dma_start(xts[c][:, 0:SL], xT.ap()[rows(c), 0:SL])
              for c in range(ND):
                nc.sync.dma_start(xts[c][:, SL:N], xT.ap()[rows(c), SL:N])
              for c in range(ND):
                nc.sync.dma_start(wa[c][:, 2 * H : H3], wT.ap()[rows(c), 2 * H : H3])
              for c in range(ND):
                nc.sync.dma_start(wa[c][:, 0:H], wT.ap()[rows(c), 0:H])

              # ---- Phase A: k and v projections (resident outputs) ----
              with (
                tc.tile_pool(name=f"qkps{rep}", bufs=3, space="PSUM") as qkps,
                tc.tile_pool(name=f"vps{rep}", bufs=2, space="PSUM") as vps,
              ):
                for ns in range(NSL):
                    ssl = slice(ns * SL, (ns + 1) * SL)
                    for hc in range(ND):
                        ps = qkps.tile([P, SL], F32, name="qkpsum", tag="qk")
                        mm_group(
                            ps, [wslice(c, H, hc) for c in range(ND)],
                            lambda c, lo, w, _ns=ns: xts[c][:, _ns * SL + lo : _ns * SL + lo + w],
                            split=proj_split,
                        )
                        evict_bias(
                            ktsb[hc][:, ssl], ps[:],
                            bcol_sb[:, ND + hc : ND + hc + 1],
                        )
                    if q_in_a:
                        for hc in range(ND):
                            ps = qkps.tile([P, SL], F32, name="qpsum", tag="qk")
                            mm_group(
                                ps, [wslice(c, 0, hc) for c in range(ND)],
                                lambda c, lo, w, _ns=ns: xts[c][:, _ns * SL + lo : _ns * SL + lo + w],
                                split=proj_split,
                            )
                            evict_bias(
                                qres[hc][:, ssl], ps[:], bcol_sb[:, hc : hc + 1]
                            )
                    for ni in range(4 * ns, 4 * ns + 4):
                        lsl = slice((ni % 4) * P + ns * SL, (ni % 4) * P + ns * SL + P)
                        pa = vps.tile([P, SL], F32, name="pa", tag="pa")
                        pb = vps.tile([P, H - SL], F32, name="pb", tag="pb")
                        mm_group(
                            pa, [xts[c][:, lsl] for c in range(ND)],
                            lambda c, lo, w: wa[c][:, 2 * H + lo : 2 * H + lo + w],
                            split=proj_split,
                        )
                        mm_group(
                            pb, [xts[c][:, lsl] for c in range(ND)],
                            lambda c, lo, w: wa[c][:, 2 * H + SL + lo : 2 * H + SL + lo + w],
                        )
                        nc.vector.tensor_tensor(
                            vsb[ni][:, 0:SL], pa[:], bvb[:, 0:SL], op=ALU.add
                        )
                        nc.vector.tensor_tensor(
                            vsb[ni][:, SL:H], pb[:], bvb[:, SL:H], op=ALU.add
                        )

              # ---- Phase B: attention (software-pipelined m-loop) ----
              with (
                tc.tile_pool(name=f"qsb_pool{rep}", bufs=2) as qsb_pool,
                tc.tile_pool(name=f"p_pool{rep}", bufs=4) as p_pool,
                tc.tile_pool(name=f"u_ps{rep}", bufs=1, space="PSUM") as u_ps,
                tc.tile_pool(name=f"sps{rep}", bufs=2, space="PSUM") as sps,
                tc.tile_pool(name=f"usb_pool{rep}", bufs=1) as usb_pool,
                tc.tile_pool(name=f"misc{rep}", bufs=1) as misc,
              ):
                for ns in range(nsl_eff):
                    if q_in_a:
                        qoff = ns * SL
                        qslice = lambda c, lo, w, _qo=qoff: qres[c][:, _qo + lo : _qo + lo + w]
                    else:
                        qsbuf = []
                        for hc in range(ND):
                            ps = sps.tile([P, SL], F32, name="s_ps", tag="s")
                            mm_group(
                                ps, [wslice(c, 0, hc) for c in range(ND)],
                                lambda c, lo, w, _ns=ns: xts[c][:, _ns * SL + lo : _ns * SL + lo + w],
                                split=proj_split,
                            )
                            qc = qsb_pool.tile([P, SL], BF16, name=f"qsb{hc}", tag=f"qsb{hc}")
                            evict_bias(qc[:], ps[:], bcol_sb[:, hc : hc + 1])
                            qsbuf.append(qc)
                        qslice = lambda c, lo, w, _q=qsbuf: _q[c][:, lo : lo + w]
                    us = [
                        u_ps.tile([P, SL], F32, name=f"u{c}", tag=f"u{c}")
                        for c in range(ND)
                    ]
                    r_sb = misc.tile([P, SL], F32, name="r_sb", tag="r_sb")

                    p_sbs = [None] * NM
                    for mi in range(nm_eff + 1):
                        if mi < nm_eff:
                            msl = slice(mi * P, (mi + 1) * P)
                            s_ps = sps.tile([P, SL], F32, name="s_ps", tag="s")
                            mm_group(
                                s_ps, [ktsb[c][:, msl] for c in range(ND)],
                                qslice,
                                split=split,
                            )
                            p_sb = p_pool.tile([P, SL], BF16, name="p_sb", tag="p")
                            nc.scalar.activation(p_sb[:], s_ps[:], AF.Exp)
                            p_sbs[mi] = p_sb
                        if mi >= 1:
                            j = mi - 1
                            pj = p_sbs[j]
                            r_ps = sps.tile([P, SL], F32, name="r_ps", tag="s")
                            mm_group(
                                r_ps, [ones128[:]],
                                lambda c, lo, w: pj[:, lo : lo + w],
                                split=split,
                            )
                            if j == 0:
                                nc.vector.tensor_copy(r_sb[:], r_ps[:])
                            else:
                                nc.vector.tensor_tensor(
                                    r_sb[:], r_ps[:], r_sb[:], op=ALU.add
                                )
                            hw2 = SL // split
                            for c in range(ND):
                                for h in range(split):
                                    lo = h * hw2
                                    nc.tensor.matmul(
                                        us[c][:, lo : lo + hw2],
                                        vsb[j][:, c * P : (c + 1) * P],
                                        pj[:, lo : lo + hw2],
                                        start=(j == 0 and h == 0),
                                        stop=(j == nm_eff - 1 and h == split - 1),
                                    )
                            p_sbs[j] = None

                    rinv = misc.tile([P, SL], F32, name="rinv", tag="rinv")
                    nc.vector.reciprocal(rinv[:], r_sb[:])

                    for c in range(ND):
                        u_sb = usb_pool.tile([P, SL], F32, name=f"usb{c}", tag=f"usb{c}")
                        nc.vector.tensor_tensor(u_sb[:], us[c][:], rinv[:], op=ALU.mult)
                        nc.gpsimd.dma_start(
                            out.ap()[rows(c), ns * SL : (ns + 1) * SL], u_sb[:]
                        )

    nc.compile()
    return nc


_NC = None

# Best-known configuration (measured 454us vs 553us for the fp32r build):
# v1 structure, every matmul operand bf16, 512-wide matmuls issued as 2x256.
BEST_KW = dict(all_bf16=True, split=2)
KERNEL_KW_HOST = dict(all_bf16=True, v2=False)


def build_best(loop_iters=None):
    return build_nc(loop_iters=loop_iters, **BEST_KW)


def make_in_maps(x: np.ndarray, W: np.ndarray, b: np.ndarray,
                 all_bf16: bool = False, v2: bool = True):
    """Per-core input maps matching the builder's DRAM tensor layout/dtypes."""
    x = np.ascontiguousarray(x, dtype=np.float32)
    W = np.ascontiguousarray(W, dtype=np.float32)
    b = np.ascontiguousarray(b, dtype=np.float32)

    wT = np.ascontiguousarray(W.T)                      # [768, 2304]
    bcol = np.ascontiguousarray(b[: 2 * H].reshape(2 * ND, P).T)  # [128, 12]

    if v2:
        import ml_dtypes

        wT = np.ascontiguousarray(wT.astype(ml_dtypes.bfloat16))
        bvb = np.ascontiguousarray(np.tile(b[2 * H :].reshape(1, H), (P, 1)))
        ones = np.ones((P, P), dtype=ml_dtypes.bfloat16)
        return [
            {
                "xT": np.ascontiguousarray(x[i].T.astype(ml_dtypes.bfloat16)),
                "wT": wT, "bcol": bcol, "bvb": bvb, "ones": ones,
            }
            for i in range(B)
        ]

    bvrow = np.ascontiguousarray(b[2 * H :].reshape(1, H))
    if all_bf16:
        import ml_dtypes

        wT = np.ascontiguousarray(wT.astype(ml_dtypes.bfloat16))
        xTs = [np.ascontiguousarray(x[i].T.astype(ml_dtypes.bfloat16)) for i in range(B)]
    else:
        xTs = [np.ascontiguousarray(x[i].T) for i in range(B)]

    return [
        {"xT": xTs[i], "wT": wT, "bcol": bcol, "bvrow": bvrow} for i in range(B)
    ]


def kernel(x: np.ndarray, W: np.ndarray, b: np.ndarray) -> np.ndarray:
    global _NC
    if _NC is None:
        _NC = build_best()

    in_maps = make_in_maps(x, W, b, **KERNEL_KW_HOST)
    res = run_bass_kernel_spmd(_NC, in_maps, core_ids=list(range(B)))
    return np.stack(
        [np.ascontiguousarray(res.results[i]["out"].T) for i in range(B)], axis=0
    )

